# revision 9
# baseline (speedup 1.0000x reference)
"""Trainium2 Bass kernel for a 12-head MHA layer with relative position bias
and a 0/1 attention mask (B=2, N=2048, C=768, H=12, d=64), sharded over 8
NeuronCores (batch x head-group parallel: core c handles batch c//4 and heads
3*(c%4) .. 3*(c%4)+2).

Device math per core (heads i = 0..2, all in transposed "T" layouts):
  qT = (Wq*s).T^T @ xT         (s = d^-0.5 folded into Wq on host)
  ST[k,q]  = kT.T @ qT         (PSUM, per 128-row k-tile)
  E[k,q]   = exp(ST + EBT)     (DVE add + ACT exp; EBT = (rel_bias + maskadd).T
                                in bf16, maskadd = -1e30 where mask==0)
  otT[d',q]= sum_k v'[k,d'] E[k,q]   (v' = [v | ones] -> row 64 = softmax denom)
  onrm     = otT[0:64] * (1/denom)   (recip via ACT ln+exp, PE outer-product
                                      broadcast across partitions)
  ytT      = pwT.T @ concat_i(onrm)  (partial projection, summed on host)
"""

import os
import numpy as np
import ml_dtypes

import concourse.bass as bass
import concourse.tile as tile
from concourse import bacc, mybir
from concourse.alu_op_type import AluOpType
from concourse.bass_utils import run_bass_kernel_spmd

AF = mybir.ActivationFunctionType
DT = mybir.dt

B, N, C, H, D = 2, 2048, 768, 12, 64
HPC = H // 4          # heads per core (8 cores = 2 batches x 4 head-groups)
NCORES = 8
SCALE = float(D) ** -0.5

LAST_RESULTS = None   # BassKernelResults of the most recent kernel() call


def _q_chunks(n):
    """Split n into <=512 free-dim chunks for matmul moving operands."""
    out = []
    o = 0
    while o < n:
        sz = min(512, n - o)
        out.append((o, sz))
        o += sz
    return out


def build_program(n=N, c_in=C, hpc=HPC, d=D, c_out=C):
    """Build the per-core Bass/Tile program. Same program runs on all cores
    (SPMD); per-core data differs via in_maps."""
    nt = n // 128                       # number of 128-row k-tiles
    qch = _q_chunks(n)
    ck = (c_in + 127) // 128            # contraction chunks over c_in
    # wqk columns: [q0|q1], [k0|k1], [q2|pad], [k2|pad] in 128-col m-chunks so
    # that each head's qT and kT live at the same partition base (0 or 64).
    n_qk_chunks = 2 * ((hpc + 1) // 2)  # 4 for hpc=3
    wqk_cols = 128 * n_qk_chunks
    wv_cols = hpc * (d + 1)             # [v_i | ones-slot] per head
    mo = c_out // 128                   # proj output row chunks

    def pc(kc):
        return min(128, c_in - 128 * kc)

    nc = bacc.Bacc("TRN2", target_bir_lowering=False, debug=False)
    xt = nc.dram_tensor("xt", [c_in, n], DT.float32, kind="ExternalInput").ap()
    wqk = nc.dram_tensor("wqk", [c_in, wqk_cols], DT.float32, kind="ExternalInput").ap()
    wv = nc.dram_tensor("wv", [c_in, wv_cols], DT.float32, kind="ExternalInput").ap()
    eb = nc.dram_tensor("eb", [hpc, n, n], DT.bfloat16, kind="ExternalInput").ap()
    pw = nc.dram_tensor("pw", [hpc * d, c_out], DT.float32, kind="ExternalInput").ap()
    yt = nc.dram_tensor("yt", [c_out, n], DT.float32, kind="ExternalOutput").ap()

    with tile.TileContext(nc) as tc:
        # ---- pools (stack allocator: xts/weights released before attn) ----
        persist = tc.alloc_tile_pool(name="persist", bufs=1)
        qkvout = tc.alloc_tile_pool(name="qkvout", bufs=1)
        loadp = tc.alloc_tile_pool(name="loadp", bufs=1)
        ps_qkv = tc.alloc_tile_pool(name="ps_qkv", bufs=4, space="PSUM")

        pw_s = persist.tile([64, hpc, c_out], DT.float32, tag="pw")
        ones_s = persist.tile([1, 128], DT.float32, tag="ones")
        nc.vector.memset(ones_s, 1.0)
        for i in range(hpc):
            nc.sync.dma_start(out=pw_s[:, i, :], in_=pw[64 * i:64 * i + 64, :])

        qk_s = qkvout.tile([128, n_qk_chunks, n], DT.float32, tag="qk")
        v_s = qkvout.tile([128, nt, wv_cols], DT.float32, tag="v")

        xts = loadp.tile([128, ck, n], DT.float32, tag="xts")
        wqk_s = loadp.tile([128, ck, wqk_cols], DT.float32, tag="wqk")
        wv_s = loadp.tile([128, ck, wv_cols], DT.float32, tag="wv")
        for kc in range(ck):
            p = pc(kc)
            nc.sync.dma_start(out=xts[:p, kc, :], in_=xt[128 * kc:128 * kc + p, :])
            nc.sync.dma_start(out=wqk_s[:p, kc, :], in_=wqk[128 * kc:128 * kc + p, :])
            nc.sync.dma_start(out=wv_s[:p, kc, :], in_=wv[128 * kc:128 * kc + p, :])

        # ---- phase B: qkT = wqk.T @ xT  -> qk_s ----
        for m in range(n_qk_chunks):
            for (fo, fs) in qch:
                ps = ps_qkv.tile([128, 512], DT.float32, tag="psqkv")
                for kc in range(ck):
                    p = pc(kc)
                    nc.tensor.matmul(
                        ps[:, :fs],
                        lhsT=wqk_s[:p, kc, 128 * m:128 * m + 128],
                        rhs=xts[:p, kc, fo:fo + fs],
                        start=(kc == 0), stop=(kc == ck - 1),
                    )
                nc.vector.tensor_copy(qk_s[:, m, fo:fo + fs], ps[:, :fs])

        # ---- phase C: v' = xT.T @ wv -> v_s (natural layout, k on partitions)
        for j in range(nt):
            ps = ps_qkv.tile([128, wv_cols], DT.float32, tag="psqkv")
            for kc in range(ck):
                p = pc(kc)
                nc.tensor.matmul(
                    ps,
                    lhsT=xts[:p, kc, 128 * j:128 * j + 128],
                    rhs=wv_s[:p, kc, :],
                    start=(kc == 0), stop=(kc == ck - 1),
                )
            nc.vector.tensor_copy(v_s[:, j, :], ps)
            # ones column for the softmax-denominator trick
            nc.vector.memset(
                v_s[:, j, :].rearrange("p (h c) -> p h c", c=d + 1)[:, :, d], 1.0)

        loadp.release()   # free xts/wqk_s/wv_s space for attention pools
        ps_qkv.release()  # free PSUM banks for the attention pools

        # ---- attention pools ----
        ebp = tc.alloc_tile_pool(name="ebp", bufs=3)
        e0p = tc.alloc_tile_pool(name="e0p", bufs=2)
        e1p = tc.alloc_tile_pool(name="e1p", bufs=2)
        normp = tc.alloc_tile_pool(name="normp", bufs=1)
        ps_st = tc.alloc_tile_pool(name="ps_st", bufs=1, space="PSUM")
        ps_ot = tc.alloc_tile_pool(name="ps_ot", bufs=1, space="PSUM")

        osum = [normp.tile([65, n], DT.float32, tag=f"osum{i}", name=f"osum{i}")
                for i in range(hpc)]

        def head_aps(i):
            base = 64 * (i % 2)
            qv = qk_s[base:base + 64, 2 * (i // 2), :]
            kv = qk_s[base:base + 64, 2 * (i // 2) + 1, :]
            return qv, kv

        # ---- phase D: per-head attention ----
        for i in range(hpc):
            qv, kv = head_aps(i)
            ot = ps_ot.tile([65, n], DT.float32, tag="ot")
            for j in range(nt):
                eb_t = ebp.tile([128, n], DT.bfloat16, tag="eb")
                nc.sync.dma_start(out=eb_t, in_=eb[i, 128 * j:128 * j + 128, :])
                st = ps_st.tile([128, n], DT.float32, tag="st")
                for (fo, fs) in qch:
                    nc.tensor.matmul(
                        st[:, fo:fo + fs],
                        lhsT=kv[:, 128 * j:128 * j + 128],
                        rhs=qv[:, fo:fo + fs],
                        start=True, stop=True,
                    )
                e0 = e0p.tile([128, n], DT.float32, tag="e0")
                nc.vector.tensor_tensor(e0, st, eb_t, AluOpType.add)
                e1 = e1p.tile([128, n], DT.float32, tag="e1")
                nc.scalar.activation(e1, e0, AF.Exp)
                for (fo, fs) in qch:
                    nc.tensor.matmul(
                        ot[:, fo:fo + fs],
                        lhsT=v_s[:, j, (d + 1) * i:(d + 1) * i + d + 1],
                        rhs=e1[:, fo:fo + fs],
                        start=(j == 0), stop=(j == nt - 1),
                    )
            nc.vector.tensor_copy(osum[i], ot)

        # ---- phase E: normalization (deferred; recip = exp(-ln(sum))) ----
        rsb = None
        for i in range(hpc):
            lnr = normp.tile([1, n], DT.float32, tag="lnr")
            nc.scalar.activation(lnr, osum[i][64:65, :], AF.Ln)
            rrow = normp.tile([1, n], DT.float32, tag="rrow")
            nc.scalar.activation(rrow, lnr, AF.Exp, scale=-1.0)
            rps = ps_st.tile([64, n], DT.float32, tag="st")
            for (fo, fs) in qch:
                nc.tensor.matmul(
                    rps[:, fo:fo + fs],
                    lhsT=ones_s[0:1, 0:64],
                    rhs=rrow[:, fo:fo + fs],
                    start=True, stop=True,
                )
            rsb = normp.tile([64, n], DT.float32, tag="rsb")
            nc.scalar.copy(rsb, rps)
            # in-place: osum[i][0:64] *= rsb
            nc.vector.tensor_tensor(osum[i][0:64, :], osum[i][0:64, :], rsb,
                                    AluOpType.mult)

        ps_ot.release()
        ps_st.release()

        # ---- phase F: partial projection ytT = pw.T @ onrm ----
        ps_pj = tc.alloc_tile_pool(name="ps_pj", bufs=2, space="PSUM")
        ytp = tc.alloc_tile_pool(name="ytp", bufs=2)
        for m in range(mo):
            ps = ps_pj.tile([128, n], DT.float32, tag="pj")
            for (fo, fs) in qch:
                for i in range(hpc):
                    nc.tensor.matmul(
                        ps[:, fo:fo + fs],
                        lhsT=pw_s[:, i, 128 * m:128 * m + 128],
                        rhs=osum[i][0:64, fo:fo + fs],
                        start=(i == 0), stop=(i == hpc - 1),
                    )
            yts = ytp.tile([128, n], DT.float32, tag="yts")
            if m % 2 == 0:
                nc.vector.tensor_copy(yts, ps)
            else:
                nc.scalar.copy(yts, ps)
            nc.sync.dma_start(out=yt[128 * m:128 * m + 128, :], in_=yts)

        ps_pj.release()
        ytp.release()
        normp.release()
        e1p.release()
        e0p.release()
        ebp.release()
        qkvout.release()
        persist.release()

    nc.compile()
    return nc


_PROG = {}


def _get_program(**kw):
    key = tuple(sorted(kw.items()))
    if key not in _PROG:
        _PROG[key] = build_program(**kw)
    return _PROG[key]


def make_in_maps(x, mask, qkv_w, qkv_b, rel_bias, proj_w):
    """Host-side shard + layout prep. Returns list of per-core input dicts."""
    x = np.asarray(x, dtype=np.float32)
    mask = np.asarray(mask)
    qkv_w = np.asarray(qkv_w, dtype=np.float32)
    qkv_b = np.asarray(qkv_b, dtype=np.float32)
    rel_bias = np.asarray(rel_bias, dtype=np.float32)
    proj_w = np.asarray(proj_w, dtype=np.float32)

    n_qk_chunks = 2 * ((HPC + 1) // 2)
    wqk_cols = 128 * n_qk_chunks
    wv_cols = HPC * (D + 1)
    has_bias = bool(np.any(qkv_b))
    c_in = C + 1 if has_bias else C

    # per-batch transposed activations
    xts = []
    for b in range(B):
        xb = x[b].T  # [C, N]
        if has_bias:
            xb = np.concatenate([xb, np.ones((1, N), np.float32)], axis=0)
        xts.append(np.ascontiguousarray(xb))

    maps = []
    for core in range(NCORES):
        b = core // 4
        heads = [HPC * (core % 4) + i for i in range(HPC)]

        wqk = np.zeros((c_in, wqk_cols), np.float32)
        wv = np.zeros((c_in, wv_cols), np.float32)
        pwm = np.zeros((HPC * D, C), np.float32)
        for i, h in enumerate(heads):
            base = 128 * (2 * (i // 2)) + 64 * (i % 2)
            wqk[:C, base:base + 64] = qkv_w[D * h:D * h + D, :].T * SCALE
            kbase = 128 * (2 * (i // 2) + 1) + 64 * (i % 2)
            wqk[:C, kbase:kbase + 64] = qkv_w[C + D * h:C + D * h + D, :].T
            wv[:C, (D + 1) * i:(D + 1) * i + D] = qkv_w[2 * C + D * h:2 * C + D * h + D, :].T
            if has_bias:
                wqk[C, base:base + 64] = qkv_b[D * h:D * h + D] * SCALE
                wqk[C, kbase:kbase + 64] = qkv_b[C + D * h:C + D * h + D]
                wv[C, (D + 1) * i:(D + 1) * i + D] = qkv_b[2 * C + D * h:2 * C + D * h + D]
            pwm[64 * i:64 * i + 64, :] = proj_w[:, D * h:D * h + D].T

        ebs = np.empty((HPC, N, N), ml_dtypes.bfloat16)
        madd = np.where(mask[b, 0] != 0, np.float32(0), np.float32(-1e30))
        for i, h in enumerate(heads):
            ebs[i] = (rel_bias[h] + madd).T.astype(ml_dtypes.bfloat16)

        maps.append({
            "xt": xts[b],
            "wqk": wqk,
            "wv": wv,
            "eb": ebs,
            "pw": pwm,
        })
    return maps, has_bias


def kernel(x, mask, qkv_w, qkv_b, rel_bias, proj_w, proj_b):
    global LAST_RESULTS
    maps, has_bias = make_in_maps(x, mask, qkv_w, qkv_b, rel_bias, proj_w)
    nc = _get_program(c_in=C + 1 if has_bias else C)

    trace = bool(os.environ.get("KERNEL_TRACE"))
    try:
        res = run_bass_kernel_spmd(
            nc, maps, list(range(NCORES)),
            trace=trace,
            trace_cores=list(range(NCORES)) if trace else None,
        )
    except Exception:
        if not trace:
            raise
        # tracing infra unavailable; rerun untraced
        os.environ["BASS_NEVER_TRACE"] = "1"
        res = run_bass_kernel_spmd(nc, maps, list(range(NCORES)), trace=False)
    LAST_RESULTS = res

    proj_b = np.asarray(proj_b, dtype=np.float32)
    out = np.empty((B, N, C), np.float32)
    for b in range(B):
        acc = res.results[4 * b]["yt"].astype(np.float32)
        for c in range(4 * b + 1, 4 * b + 4):
            acc = acc + res.results[c]["yt"]
        out[b] = acc.T + proj_b[None, :]
    return out


# revision 18
# speedup vs baseline: 2.9673x; 2.9673x over previous
"""Trainium2 Bass kernel for a 12-head MHA layer with relative position bias
and a 0/1 attention mask (B=2, N=2048, C=768, H=12, d=64), sharded over 8
NeuronCores (batch x head-group parallel: core c handles batch c//4 and heads
3*(c%4) .. 3*(c%4)+2).

Device math per core (heads i = 0..2, all in transposed "T" layouts):
  qT = (Wq*s).T^T @ xT         (s = d^-0.5 folded into Wq on host)
  ST[k,q]  = kT.T @ qT         (PSUM, per 128-row k-tile)
  E[k,q]   = exp(ST) * EBT     (ACT exp to bf16 + DVE 2x-mode mult;
                                EBT = (exp(rel_bias)*mask).T in bf16)
  otT[d',q]= sum_k v'[k,d'] E[k,q]   (v' = [v | ones] -> row 64 = softmax denom)
  onrm     = otT[0:64] * (1/denom)   (recip via ACT ln+exp, PE outer-product
                                      broadcast across partitions)
  ytT      = pwT.T @ concat_i(onrm)  (partial projection, summed on host)
"""

import os
import numpy as np
import ml_dtypes

import concourse.bass as bass
import concourse.tile as tile
from concourse.tile import add_dep_helper
from concourse import bacc, mybir
from concourse.alu_op_type import AluOpType
from concourse.bass_utils import run_bass_kernel_spmd

AF = mybir.ActivationFunctionType
DT = mybir.dt
F32R = mybir.dt.float32r

B, N, C, H, D = 2, 2048, 768, 12, 64
HPC = H // 4          # heads per core (8 cores = 2 batches x 4 head-groups)
NCORES = 8
SCALE = float(D) ** -0.5

LAST_RESULTS = None   # BassKernelResults of the most recent kernel() call


def _q_chunks(n):
    """Split n into <=512 free-dim chunks for matmul moving operands."""
    out = []
    o = 0
    while o < n:
        sz = min(512, n - o)
        out.append((o, sz))
        o += sz
    return out


def build_program(n=N, c_in=C, hpc=HPC, d=D, c_out=C):
    """Build the per-core Bass/Tile program. Same program runs on all cores
    (SPMD); per-core data differs via in_maps."""
    nt = n // 128                       # number of 128-row k-tiles
    qch = _q_chunks(n)
    ck = (c_in + 127) // 128            # contraction chunks over c_in
    # wqk columns: [q0|q1], [k0|k1], [q2|pad], [k2|pad] in 128-col m-chunks so
    # that each head's qT and kT live at the same partition base (0 or 64).
    n_qk_chunks = 2 * ((hpc + 1) // 2)  # 4 for hpc=3
    wqk_cols = 128 * n_qk_chunks
    wv_cols = hpc * (d + 2)             # [v_i | ones | pad] per head (even stride for fp32r)
    mo = c_out // 128                   # proj output row chunks

    def pc(kc):
        return min(128, c_in - 128 * kc)

    nc = bacc.Bacc("TRN2", target_bir_lowering=False, debug=False)
    xt = nc.dram_tensor("xt", [c_in, n], F32R, kind="ExternalInput").ap()
    wqk = nc.dram_tensor("wqk", [c_in, wqk_cols], F32R, kind="ExternalInput").ap()
    wv = nc.dram_tensor("wv", [c_in, wv_cols], F32R, kind="ExternalInput").ap()
    eb = nc.dram_tensor("eb", [hpc, n, n], DT.bfloat16, kind="ExternalInput").ap()
    pw = nc.dram_tensor("pw", [hpc * d, c_out], F32R, kind="ExternalInput").ap()
    yt = nc.dram_tensor("yt", [c_out, n], DT.float32, kind="ExternalOutput").ap()

    with tile.TileContext(nc) as tc:
        # ---- pools (stack allocator: xts/weights released before attn) ----
        persist = tc.alloc_tile_pool(name="persist", bufs=1)
        qkvout = tc.alloc_tile_pool(name="qkvout", bufs=1)
        loadp = tc.alloc_tile_pool(name="loadp", bufs=1)
        ps_qkv = tc.alloc_tile_pool(name="ps_qkv", bufs=4, space="PSUM")

        pw_s = persist.tile([64, hpc, c_out], F32R, tag="pw")
        ones_s = persist.tile([1, 128], DT.float32, tag="ones")
        nc.vector.memset(ones_s, 1.0)
        ones3 = persist.tile([128, hpc], DT.float32, tag="ones3")
        nc.vector.memset(ones3, 1.0)
        for i in range(hpc):
            nc.sync.dma_start(out=pw_s[:, i, :], in_=pw[64 * i:64 * i + 64, :])

        qk_s = qkvout.tile([128, n_qk_chunks, n], DT.bfloat16, tag="qk")
        v_s = qkvout.tile([128, nt, wv_cols], DT.bfloat16, tag="v")

        xts = loadp.tile([128, ck, n], F32R, tag="xts")
        wqk_s = loadp.tile([128, ck, wqk_cols], F32R, tag="wqk")
        wv_s = loadp.tile([128, ck, wv_cols], F32R, tag="wv")
        for kc in range(ck):
            p = pc(kc)
            nc.sync.dma_start(out=xts[:p, kc, :], in_=xt[128 * kc:128 * kc + p, :])
            nc.sync.dma_start(out=wqk_s[:p, kc, :], in_=wqk[128 * kc:128 * kc + p, :])
            nc.sync.dma_start(out=wv_s[:p, kc, :], in_=wv[128 * kc:128 * kc + p, :])

        # ---- phase B: qkT = wqk.T @ xT  -> qk_s ----
        # (m-chunks 0/1 first: they unblock head 0's attention; v' phase C sits
        # between so PE order matches the consumer order)
        def emit_qk_chunk(m):
            for (fo, fs) in qch:
                ps = ps_qkv.tile([128, 512], DT.float32, tag="psqkv", name=f"psB{m}")
                for kc in range(ck):
                    p = pc(kc)
                    nc.tensor.matmul(
                        ps[:, :fs],
                        lhsT=wqk_s[:p, kc, 128 * m:128 * m + 128],
                        rhs=xts[:p, kc, fo:fo + fs],
                        start=(kc == 0), stop=(kc == ck - 1),
                    )
                nc.vector.tensor_copy(qk_s[:, m, fo:fo + fs], ps[:, :fs])

        for m in range(min(2, n_qk_chunks)):
            emit_qk_chunk(m)

        # ---- phase C: v' = xT.T @ wv -> v_s (natural layout, k on partitions)
        for j in range(nt):
            ps = ps_qkv.tile([128, wv_cols], DT.float32, tag="psqkv")
            for kc in range(ck):
                p = pc(kc)
                nc.tensor.matmul(
                    ps,
                    lhsT=xts[:p, kc, 128 * j:128 * j + 128],
                    rhs=wv_s[:p, kc, :],
                    start=(kc == 0), stop=(kc == ck - 1),
                )
            nc.vector.tensor_copy(v_s[:, j, :], ps)
            # ones column for the softmax-denominator trick (fp32r needs a
            # rounding producer, so copy from an fp32 ones scratch)
            nc.vector.tensor_copy(
                v_s[:, j, :].rearrange("p (h c) -> p h c", c=d + 2)[:, :, d],
                ones3)

        for m in range(min(2, n_qk_chunks), n_qk_chunks):
            emit_qk_chunk(m)

        loadp.release()   # free xts/wqk_s/wv_s space for attention pools
        ps_qkv.release()  # free PSUM banks for the attention pools

        # ---- attention pools ----
        ebp = tc.alloc_tile_pool(name="ebp", bufs=3)
        e0p = tc.alloc_tile_pool(name="e0p", bufs=2)
        e1p = tc.alloc_tile_pool(name="e1p", bufs=2)
        normp = tc.alloc_tile_pool(name="normp", bufs=1)
        ps_st = tc.alloc_tile_pool(name="ps_st", bufs=2, space="PSUM")
        ps_ot = tc.alloc_tile_pool(name="ps_ot", bufs=1, space="PSUM")

        osum = [normp.tile([66, n], F32R, tag=f"osum{i}", name=f"osum{i}")
                for i in range(hpc)]

        def head_aps(i):
            base = 64 * (i % 2)
            qv = qk_s[base:base + 64, 2 * (i // 2), :]
            kv = qk_s[base:base + 64, 2 * (i // 2) + 1, :]
            return qv, kv

        # ---- phase D: per-head attention ----
        # ST is computed in q-halves with a double-buffered 2-bank PSUM tile so
        # the PE never stalls on the ACT exp drain (keeps HAM at 2.4 GHz).
        hn = min(n, 1024)
        hch = _q_chunks(hn)
        last_exp = None
        for i in range(hpc):
            qv, kv = head_aps(i)
            ot = ps_ot.tile([66, n], DT.float32, tag="ot")
            for j in range(nt):
                eb_t = ebp.tile([128, n], DT.bfloat16, tag="eb")
                nc.sync.dma_start(out=eb_t, in_=eb[i, 128 * j:128 * j + 128, :])
                for h2 in range(n // hn):
                    ho = h2 * hn
                    st = ps_st.tile([128, hn], DT.float32, tag="st")
                    for (fo, fs) in hch:
                        nc.tensor.matmul(
                            st[:, fo:fo + fs],
                            lhsT=kv[:, 128 * j:128 * j + 128],
                            rhs=qv[:, ho + fo:ho + fo + fs],
                            start=True, stop=True,
                        )
                    e0 = e0p.tile([128, hn], DT.bfloat16, tag="e0")
                    last_exp = nc.scalar.activation(e0, st, AF.Exp)
                    e1 = e1p.tile([128, hn], DT.bfloat16, tag="e1")
                    nc.vector.tensor_tensor(e1, e0, eb_t[:, ho:ho + hn],
                                            AluOpType.mult)
                    for (fo, fs) in hch:
                        nc.tensor.matmul(
                            ot[:, ho + fo:ho + fo + fs],
                            lhsT=v_s[:, j, (d + 2) * i:(d + 2) * i + d + 2],
                            rhs=e1[:, fo:fo + fs],
                            start=(j == 0), stop=(j == nt - 1),
                        )
            nc.vector.tensor_copy(osum[i], ot)

        # ---- phase E: normalization (deferred; recip = exp(-ln(sum))) ----
        # Ln's batched before Exp's: keeps ACT table switches to a minimum.
        lnrs = [normp.tile([1, n], DT.float32, tag=f"lnr{i}", name=f"lnr{i}")
                for i in range(hpc)]
        rrows = [normp.tile([1, n], DT.float32, tag=f"rrow{i}", name=f"rrow{i}")
                 for i in range(hpc)]
        for i in range(hpc):
            ln_inst = nc.scalar.activation(lnrs[i], osum[i][64:65, :], AF.Ln)
            if last_exp is not None:
                # keep all Ln's after the exps: avoids ACT table-set thrash
                add_dep_helper(ln_inst.ins, last_exp.ins, sync=False,
                               reason="act-table ordering")
        for i in range(hpc):
            nc.scalar.activation(rrows[i], lnrs[i], AF.Exp, scale=-1.0)
        for i in range(hpc):
            rrow = rrows[i]
            rsb = normp.tile([64, n], DT.float32, tag="rsb")
            for h2 in range(n // hn):
                ho = h2 * hn
                rps = ps_st.tile([64, hn], DT.float32, tag="st", name="rps")
                for (fo, fs) in hch:
                    nc.tensor.matmul(
                        rps[:, fo:fo + fs],
                        lhsT=ones_s[0:1, 0:64],
                        rhs=rrow[:, ho + fo:ho + fo + fs],
                        start=True, stop=True,
                    )
                nc.vector.tensor_copy(rsb[:, ho:ho + hn], rps)
            # in-place: osum[i][0:64] *= rsb
            nc.vector.tensor_tensor(osum[i][0:64, :], osum[i][0:64, :], rsb,
                                    AluOpType.mult)

        ps_ot.release()
        ps_st.release()

        # ---- phase F: partial projection ytT = pw.T @ onrm ----
        ps_pj = tc.alloc_tile_pool(name="ps_pj", bufs=2, space="PSUM")
        ytp = tc.alloc_tile_pool(name="ytp", bufs=2)
        for m in range(mo):
            ps = ps_pj.tile([128, n], DT.float32, tag="pj")
            for (fo, fs) in qch:
                for i in range(hpc):
                    nc.tensor.matmul(
                        ps[:, fo:fo + fs],
                        lhsT=pw_s[:, i, 128 * m:128 * m + 128],
                        rhs=osum[i][0:64, fo:fo + fs],
                        start=(i == 0), stop=(i == hpc - 1),
                    )
            yts = ytp.tile([128, n], DT.float32, tag="yts")
            nc.vector.tensor_copy(yts, ps)
            nc.sync.dma_start(out=yt[128 * m:128 * m + 128, :], in_=yts)

        ps_pj.release()
        ytp.release()
        normp.release()
        e1p.release()
        e0p.release()
        ebp.release()
        qkvout.release()
        persist.release()

    nc.compile()
    return nc


_PROG = {}


def _get_program(**kw):
    key = tuple(sorted(kw.items()))
    if key not in _PROG:
        _PROG[key] = build_program(**kw)
    return _PROG[key]


def make_in_maps(x, mask, qkv_w, qkv_b, rel_bias, proj_w):
    """Host-side shard + layout prep. Returns list of per-core input dicts."""
    x = np.asarray(x, dtype=np.float32)
    mask = np.asarray(mask)
    qkv_w = np.asarray(qkv_w, dtype=np.float32)
    qkv_b = np.asarray(qkv_b, dtype=np.float32)
    rel_bias = np.asarray(rel_bias, dtype=np.float32)
    proj_w = np.asarray(proj_w, dtype=np.float32)

    n_qk_chunks = 2 * ((HPC + 1) // 2)
    wqk_cols = 128 * n_qk_chunks
    wv_cols = HPC * (D + 2)
    has_bias = bool(np.any(qkv_b))
    c_in = C + 1 if has_bias else C

    # per-batch transposed activations
    xts = []
    for b in range(B):
        xb = x[b].T  # [C, N]
        if has_bias:
            xb = np.concatenate([xb, np.ones((1, N), np.float32)], axis=0)
        xts.append(np.ascontiguousarray(xb))

    maps = []
    for core in range(NCORES):
        b = core // 4
        heads = [HPC * (core % 4) + i for i in range(HPC)]

        wqk = np.zeros((c_in, wqk_cols), np.float32)
        wv = np.zeros((c_in, wv_cols), np.float32)
        pwm = np.zeros((HPC * D, C), np.float32)
        for i, h in enumerate(heads):
            base = 128 * (2 * (i // 2)) + 64 * (i % 2)
            wqk[:C, base:base + 64] = qkv_w[D * h:D * h + D, :].T * SCALE
            kbase = 128 * (2 * (i // 2) + 1) + 64 * (i % 2)
            wqk[:C, kbase:kbase + 64] = qkv_w[C + D * h:C + D * h + D, :].T
            wv[:C, (D + 2) * i:(D + 2) * i + D] = qkv_w[2 * C + D * h:2 * C + D * h + D, :].T
            if has_bias:
                wqk[C, base:base + 64] = qkv_b[D * h:D * h + D] * SCALE
                wqk[C, kbase:kbase + 64] = qkv_b[C + D * h:C + D * h + D]
                wv[C, (D + 2) * i:(D + 2) * i + D] = qkv_b[2 * C + D * h:2 * C + D * h + D]
            pwm[64 * i:64 * i + 64, :] = proj_w[:, D * h:D * h + D].T

        ebs = np.empty((HPC, N, N), ml_dtypes.bfloat16)
        mb = (mask[b, 0] != 0)
        for i, h in enumerate(heads):
            ebs[i] = (np.exp(rel_bias[h]) * mb).T.astype(ml_dtypes.bfloat16)

        maps.append({
            "xt": xts[b],
            "wqk": wqk,
            "wv": wv,
            "eb": ebs,
            "pw": pwm,
        })
    return maps, has_bias


def kernel(x, mask, qkv_w, qkv_b, rel_bias, proj_w, proj_b):
    global LAST_RESULTS
    maps, has_bias = make_in_maps(x, mask, qkv_w, qkv_b, rel_bias, proj_w)
    nc = _get_program(c_in=C + 1 if has_bias else C)

    trace = bool(os.environ.get("KERNEL_TRACE"))
    try:
        res = run_bass_kernel_spmd(
            nc, maps, list(range(NCORES)),
            trace=trace,
            trace_cores=list(range(NCORES)) if trace else None,
        )
    except Exception:
        if not trace:
            raise
        # tracing infra unavailable; rerun untraced
        os.environ["BASS_NEVER_TRACE"] = "1"
        res = run_bass_kernel_spmd(nc, maps, list(range(NCORES)), trace=False)
    LAST_RESULTS = res

    proj_b = np.asarray(proj_b, dtype=np.float32)
    out = np.empty((B, N, C), np.float32)
    for b in range(B):
        acc = res.results[4 * b]["yt"].astype(np.float32)
        for c in range(4 * b + 1, 4 * b + 4):
            acc = acc + res.results[c]["yt"]
        out[b] = acc.T + proj_b[None, :]
    return out
